# revision 8
# baseline (speedup 1.0000x reference)
"""MHSA3D Trainium2 kernel: 8-way head-parallel flash-style attention.

Problem (hardcoded): B=1, C=128, D=H=W=16 -> N=4096 tokens, 8 heads,
dh=16, dv=128.  Each of the 8 NeuronCores computes one head end-to-end:
qkv projection (its head's slice), S^T = k''^T q'' logits in [j, i]
layout, exp on ScalarE (no max subtraction -- fp32 exp cannot overflow
for this data), PV accumulation with an appended ones-column producing
the softmax denominator, then normalize.

The ScalarE exp stream is the roofline (N^2/128 lanes / 1.2 GHz =
109 us/core); everything is organized to keep that stream dense:
- exp tiles alternate [128,2048] / [128,1536] (4+3 PSUM banks,
  single-buffered each => hardware double-buffering across the pair),
  amortizing the ~293 ns per-ACTIVATE fixed cost over 9 instead of 16
  instructions per 512-column eighth.
- The 8th PSUM bank holds the qkv projection staging and the PV
  accumulator (17 rows: 16 v rows + denominator).
- qk contraction zero-padded to K=96 to keep the PE HAM activity
  monitor warm (2.4 GHz); q'/k'' fp16, P/v bf16.
- v^T is built by one xbar-transpose DMA per 512-column chunk
  (v [16,512] -> [128,4,16]) instead of 32 PE matmuls.
- Softmax reciprocal via the custom-DVE reciprocal_approx_fast
  (~5x faster than DVE reciprocal); recip broadcast via a tiny
  ones-matmul into spare partitions of the accumulator bank.
- PV for group g is emitted 3 groups late so the PE FIFO never
  head-of-line blocks on the exp; per-eighth normalize tails are
  deferred into the next eighth.

Host side: fold the 1/sqrt(dh) scale into wq/bq, fold b_k into the
positional-embedding plane, slice per-head weights, run the SPMD
program on cores 0-7, and concatenate the per-head [16, N] outputs.
"""

import numpy as np

NHEADS = 8
DV = 128
DH = DV // NHEADS  # 16
C = 128
N = 4096
ECOLS = 512        # i-columns handled per output tile ("eighth")
NE = N // ECOLS    # 8
JW = 128           # keys per j-block
NJB = N // JW      # 32
VS = 48            # vaugT per-block stride (ones | 31 zeros | 16 v)
LAG = 3            # PV groups trail the exp stream by this many groups

# j-block grouping per eighth: alternating 4-block (2048-col) and
# 3-block (1536-col) exp tiles; 4+3 PSUM banks double-buffer.
GSIZES = [4, 3, 4, 3, 4, 3, 4, 3, 4]
GROUPS = []
_j = 0
for _s in GSIZES:
    GROUPS.append(tuple(range(_j, _j + _s)))
    _j += _s
assert _j == NJB

_compiled = None


def _build_program():
    import concourse.bacc as bacc
    import concourse.mybir as mybir
    import concourse.tile as tile

    f32 = mybir.dt.float32
    bf16 = mybir.dt.bfloat16
    fp16 = mybir.dt.float16
    EXP = mybir.ActivationFunctionType.Exp
    ADD = mybir.AluOpType.add
    MULT = mybir.AluOpType.mult

    nc = bacc.Bacc("TRN2", target_bir_lowering=False, debug=False,
                   num_devices=NHEADS)

    x_d = nc.dram_tensor("x", [C, N], fp16, kind="ExternalInput")
    # w cols: 0-15 wq*scale, 16-31 wk, 32-47 wv
    w_d = nc.dram_tensor("w", [C, 48], fp16, kind="ExternalInput")
    # bias plane rows: 0-15 bq*scale (bcast), 16-31 bk+emb
    b_d = nc.dram_tensor("bias", [32, N], f32, kind="ExternalInput")
    bv_d = nc.dram_tensor("bv", [DH, 1], f32, kind="ExternalInput")
    o_d = nc.dram_tensor("out", [DH, N], f32, kind="ExternalOutput")

    with tile.TileContext(nc) as tc:
        with (
            tc.tile_pool(name="const", bufs=1) as const,
            tc.tile_pool(name="pt", bufs=3) as ptp,
            tc.tile_pool(name="o", bufs=3) as op,
            tc.tile_pool(name="stA", bufs=1, space="PSUM") as stAp,
            tc.tile_pool(name="stB", bufs=1, space="PSUM") as stBp,
            tc.tile_pool(name="acc", bufs=1, space="PSUM") as accp,
        ):
            x_s = const.tile([C, N], fp16)
            w_s = const.tile([C, 48], fp16)
            bv_s = const.tile([DH, 1], f32)
            biasf = const.tile([32, N], f32)
            # qz rows: 0-15 q''; 16-95 zero.  kz rows: 0-15 k''; 16-95
            # exact zero (mask the qz zero rows in the K=96 contraction).
            qzt = [const.tile([96, 512], fp16, name=f"qzt{c}")
                   for c in range(8)]
            kzt = [const.tile([96, 512], fp16, name=f"kzt{c}")
                   for c in range(8)]
            vaugT = const.tile([128, VS * NJB], bf16)
            v_s = const.tile([DH, N], bf16)
            ones16 = const.tile([1, DH], f32)
            zsb = const.tile([96, 512], fp16)
            zerob = const.tile([128, 1], f32)
            scratch1 = const.tile([128, 1], f32)
            # single PSUM bank: qkv projection staging, then the PV
            # accumulator rows 0-16 + recip-broadcast rows 64-79.
            acc_full = accp.tile([128, 512], f32)

            # --- startup: memsets, DMAs, exp-table warm ---
            nc.vector.memset(zsb[:], 0.0)
            nc.gpsimd.memset(zerob[:], 0.0)
            nc.gpsimd.memset(ones16[:], 1.0)
            # Warm the exp table set while DMAs run.
            nc.scalar.activation(scratch1[:], zerob[:], EXP, bias=zerob[:])

            # Per-block layout: col 0 = ones (denominator -> acc row 0,
            # partition-0-aligned for the DVE reciprocal), cols 1-31
            # zero, cols 32-47 = v^T (32B-aligned xbar-transpose dsts).
            va3 = vaugT[:].rearrange("p (c s) -> p c s", s=VS)
            nc.vector.memset(va3[:, :, 0:1], 1.0)
            nc.vector.memset(va3[:, :, 1:2 * DH], 0.0)

            nc.sync.dma_start(x_s[:, 0:1024], x_d.ap()[:, 0:1024])
            # Zero pads for chunk 0 gate the first qk matmul: issue them
            # on sync right behind the first x chunk.
            nc.sync.dma_start(kzt[0][DH:96, :], zsb[0:80, :])
            nc.sync.dma_start(qzt[0][2 * DH:96, :], zsb[0:64, :])
            for q4 in range(1, 4):
                q4s = slice(q4 * 1024, (q4 + 1) * 1024)
                nc.sync.dma_start(x_s[:, q4s], x_d.ap()[:, q4s])
            nc.gpsimd.dma_start(w_s[:], w_d.ap())
            nc.gpsimd.dma_start(biasf[:], b_d.ap())
            nc.gpsimd.dma_start(bv_s[:], bv_d.ap())
            for c in range(1, 8):
                nc.gpsimd.dma_start(kzt[c][DH:96, :], zsb[0:80, :])
            for c in range(1, 8):
                nc.gpsimd.dma_start(qzt[c][2 * DH:96, :], zsb[0:64, :])

            # --- qkv projection: one K=128 M=48 matmul per 512-col chunk.
            # Chunk 0 stages in acc_full[0:48] (freed early for the PV
            # accumulator); chunks 1-7 serialize through acc_full[64:112].
            for c in range(8):
                cs = slice(c * 512, (c + 1) * 512)
                if c == 0:
                    ps = acc_full[0:48, :]
                    tp = (0, 0)
                else:
                    ps = acc_full[64:112, :]
                    tp = (0, 64)
                nc.tensor.matmul(ps, lhsT=w_s[:, 0:48], rhs=x_s[:, cs],
                                 start=True, stop=True, tile_position=tp)
                nc.vector.tensor_tensor(qzt[c][0:2 * DH, :], ps[0:2 * DH, :],
                                        biasf[:, cs], ADD)
                nc.vector.tensor_copy(v_s[:, cs], ps[2 * DH:3 * DH, :])
                nc.gpsimd.dma_start(kzt[c][0:DH, :], qzt[c][DH:2 * DH, :])
                # v^T for this chunk via the xbar transpose engine.
                for jb in range(4 * c, 4 * c + 4):
                    nc.sync.dma_start_transpose(
                        va3[:, jb, 2 * DH:3 * DH],
                        v_s[:, jb * JW:(jb + 1) * JW])

            def make_pv(pt, jbs, acc, start, stop):
                def emit():
                    for t, jb in enumerate(jbs):
                        nc.tensor.matmul(
                            acc,
                            lhsT=vaugT[:, VS * jb:VS * (jb + 1)],
                            rhs=pt[:, 512 * t:512 * (t + 1)],
                            start=(start and t == 0),
                            stop=(stop and t == len(jbs) - 1),
                            skip_group_check=True)
                return emit

            def make_tail_a(acc):
                o17 = op.tile([3 * DH, ECOLS], f32, tag="o17")
                r = op.tile([1, ECOLS], f32, tag="r")

                def emit():
                    nc.vector.tensor_copy(o17[:], acc)
                    nc.vector.reciprocal_approx_fast(r[:], o17[0:1, :])
                return emit, o17, r

            def make_tail_b(o17, r, es):
                def emit():
                    # broadcast r across 16 partitions via a ones-matmul
                    # into spare partitions of the accumulator bank.
                    bc = acc_full[64:64 + DH, :]
                    nc.tensor.matmul(bc, lhsT=ones16[:], rhs=r[:],
                                     start=True, stop=True,
                                     tile_position=(0, 64),
                                     skip_group_check=True)
                    ost = op.tile([DH, ECOLS], f32, tag="ost")
                    nc.vector.tensor_tensor(ost[:], o17[2 * DH:3 * DH, :], bc, MULT)
                    nc.vector.tensor_scalar_add(ost[:], ost[:], bv_s[:])
                    nc.sync.dma_start(o_d.ap()[:, es], ost[:])
                return emit

            from collections import deque
            pend = deque()
            pending_a = None
            pending_b = None
            acc48 = acc_full[0:3 * DH, :]
            for e in range(NE):
                es = slice(e * ECOLS, (e + 1) * ECOLS)
                lag = LAG if e < NE - 1 else 1
                for gi, jbs in enumerate(GROUPS):
                    njb = len(jbs)
                    fw = 512 * njb
                    if njb == 4:
                        st = stAp.tile([128, 2048], f32, tag="A")
                    else:
                        st = stBp.tile([128, 1536], f32, tag="B")
                    for t, jb in enumerate(jbs):
                        kc = kzt[jb // 4][:, (jb % 4) * JW:(jb % 4 + 1) * JW]
                        nc.tensor.matmul(st[:, 512 * t:512 * (t + 1)],
                                         lhsT=kc, rhs=qzt[e][:],
                                         start=True, stop=True)
                    pt = ptp.tile([128, fw], bf16, tag=("ptA" if njb == 4
                                                        else "ptB"))
                    nc.scalar.activation(pt[:], st[:], EXP, bias=zerob[:])
                    while len(pend) >= lag:
                        pend.popleft()()
                    # The prior eighth's accumulator must be copied out
                    # (tail_a) before this eighth's start=True PV clears
                    # it.  With lag==1 that clear is emitted at gi==1, so
                    # fire tail_a at gi==0; with lag==3 it is emitted at
                    # gi==3 and gi==2 keeps the copy off the PE's heels.
                    tail_a_gi = 2 if lag >= 3 else 0
                    if pending_a is not None and gi == tail_a_gi:
                        pending_a()
                        pending_a = None
                    if pending_b is not None and gi == 6:
                        pending_b()
                        pending_b = None
                    pend.append(make_pv(pt, jbs, acc48,
                                        start=(gi == 0),
                                        stop=(gi == len(GROUPS) - 1)))
                while pend and e == NE - 1:
                    pend.popleft()()
                emit_a, o17, r = make_tail_a(acc48)
                pending_a = emit_a
                if pending_b is not None:
                    pending_b()
                pending_b = make_tail_b(o17, r, es)
            while pend:
                pend.popleft()()
            pending_a()
            pending_b()

    nc.compile()
    return nc


def _get_program():
    global _compiled
    if _compiled is None:
        _compiled = _build_program()
    return _compiled


def _prepare_core_inputs(x, w_qkv, b_qkv, emb_d, emb_h, emb_w):
    x2 = np.ascontiguousarray(
        np.asarray(x, np.float32).reshape(C, N)).astype(np.float16)
    w_qkv = np.asarray(w_qkv, np.float32)
    b_qkv = np.asarray(b_qkv, np.float32)
    scale = DH ** -0.5
    emb = (np.asarray(emb_d, np.float32)
           + np.asarray(emb_h, np.float32)
           + np.asarray(emb_w, np.float32)).reshape(DH, N)
    in_maps = []
    for h in range(NHEADS):
        qc = slice(h * DH, (h + 1) * DH)
        kc = slice(DV + h * DH, DV + (h + 1) * DH)
        vc = slice(2 * DV + h * DH, 2 * DV + (h + 1) * DH)
        w = np.empty((C, 48), np.float32)
        w[:, 0:16] = w_qkv[:, qc] * scale
        w[:, 16:32] = w_qkv[:, kc]
        w[:, 32:48] = w_qkv[:, vc]
        w = w.astype(np.float16)
        bias = np.empty((32, N), np.float32)
        bias[0:16, :] = (b_qkv[qc] * scale)[:, None]
        bias[16:32, :] = b_qkv[kc][:, None] + emb
        bv = np.ascontiguousarray(b_qkv[vc][:, None])
        in_maps.append({"x": x2, "w": w, "bias": bias, "bv": bv})
    return in_maps


def kernel(x, w_qkv, b_qkv, emb_d, emb_h, emb_w):
    from concourse.bass_utils import run_bass_kernel_spmd

    nc = _get_program()
    in_maps = _prepare_core_inputs(x, w_qkv, b_qkv, emb_d, emb_h, emb_w)
    res = run_bass_kernel_spmd(nc, in_maps, list(range(NHEADS)))
    out = np.empty((DV, N), np.float32)
    for h in range(NHEADS):
        out[h * DH:(h + 1) * DH, :] = res.results[h]["out"]
    return out.reshape(1, DV, 16, 16, 16)


# revision 10
# speedup vs baseline: 1.1625x; 1.1625x over previous
"""MHSA3D Trainium2 kernel: 8-way head-parallel flash-style attention.

Problem (hardcoded): B=1, C=128, D=H=W=16 -> N=4096 tokens, 8 heads,
dh=16, dv=128.  Each of the 8 NeuronCores computes one head end-to-end:
qkv projection (its head's slice), S^T = k''^T q'' logits in [j, i]
layout, exp on ScalarE (no max subtraction -- fp32 exp cannot overflow
for this data), PV accumulation with an appended ones-column producing
the softmax denominator, then normalize.

The ScalarE exp stream is the roofline (N^2/128 lanes / 1.2 GHz =
109 us/core); everything is organized to keep that stream dense:
- exp tiles alternate [128,2048] / [128,1536] (4+3 PSUM banks,
  single-buffered each => hardware double-buffering across the pair),
  amortizing the ~293 ns per-ACTIVATE fixed cost over 9 instead of 16
  instructions per 512-column eighth.
- The 8th PSUM bank holds the qkv projection staging and the PV
  accumulator (17 rows: 16 v rows + denominator).
- qk contraction zero-padded to K=96 to keep the PE HAM activity
  monitor warm (2.4 GHz); q'/k'' fp16, P/v bf16.
- v^T is built by one xbar-transpose DMA per 512-column chunk
  (v [16,512] -> [128,4,16]) instead of 32 PE matmuls.
- Softmax reciprocal via the custom-DVE reciprocal_approx_fast
  (~5x faster than DVE reciprocal); recip broadcast via a tiny
  ones-matmul into spare partitions of the accumulator bank.
- PV for group g is emitted 3 groups late so the PE FIFO never
  head-of-line blocks on the exp; per-eighth normalize tails are
  deferred into the next eighth.

Host side: fold the 1/sqrt(dh) scale into wq/bq, fold b_k into the
positional-embedding plane, slice per-head weights, run the SPMD
program on cores 0-7, and concatenate the per-head [16, N] outputs.
"""

import numpy as np

NHEADS = 8
DV = 128
DH = DV // NHEADS  # 16
C = 128
N = 4096
ECOLS = 512        # i-columns handled per output tile ("eighth")
NE = N // ECOLS    # 8
JW = 128           # keys per j-block
NJB = N // JW      # 32
VS = 48            # vaugT per-block stride (ones | 31 zeros | 16 v)
LAG = 3            # PV groups trail the exp stream by this many groups

# j-block grouping per eighth: alternating 4-block (2048-col) and
# 3-block (1536-col) exp tiles; 4+3 PSUM banks double-buffer.
GSIZES = [4, 3, 4, 3, 4, 3, 4, 3, 4]
GROUPS = []
_j = 0
for _s in GSIZES:
    GROUPS.append(tuple(range(_j, _j + _s)))
    _j += _s
assert _j == NJB

_compiled = None


def _build_program():
    import concourse.bacc as bacc
    import concourse.mybir as mybir
    import concourse.tile as tile

    f32 = mybir.dt.float32
    bf16 = mybir.dt.bfloat16
    fp16 = mybir.dt.float16
    EXP = mybir.ActivationFunctionType.Exp
    ADD = mybir.AluOpType.add
    MULT = mybir.AluOpType.mult

    nc = bacc.Bacc("TRN2", target_bir_lowering=False, debug=False,
                   num_devices=NHEADS)

    x_d = nc.dram_tensor("x", [C, N], fp16, kind="ExternalInput")
    # w cols: 0-15 wq*scale, 16-31 wk, 32-47 wv
    w_d = nc.dram_tensor("w", [C, 48], fp16, kind="ExternalInput")
    # bias plane rows: 0-15 bq*scale (bcast), 16-31 bk+emb, 32-47 zero
    b_d = nc.dram_tensor("bias", [48, N], f32, kind="ExternalInput")
    bv_d = nc.dram_tensor("bv", [DH, 1], f32, kind="ExternalInput")
    o_d = nc.dram_tensor("out", [DH, N], f32, kind="ExternalOutput")

    with tile.TileContext(nc) as tc:
        with (
            tc.tile_pool(name="const", bufs=1) as const,
            tc.tile_pool(name="pt", bufs=3) as ptp,
            tc.tile_pool(name="o", bufs=3) as op,
            tc.tile_pool(name="stA", bufs=1, space="PSUM") as stAp,
            tc.tile_pool(name="stB", bufs=1, space="PSUM") as stBp,
            tc.tile_pool(name="acc", bufs=1, space="PSUM") as accp,
        ):
            x_s = const.tile([C, N], fp16)
            w_s = const.tile([C, 48], fp16)
            bv_s = const.tile([DH, 1], f32)
            biasf = const.tile([48, N], f32)
            # qz rows: 0-15 q''; 16-31 k''; 32-47 v (all three from one
            # DVE add -- rows 16-47 are masked by the kz zero rows in the
            # K=96 contraction, and v doubles as the transpose source);
            # 48-95 zero.  kz rows: 0-15 k''; 16-95 exact zero.
            qzt = [const.tile([96, 512], fp16, name=f"qzt{c}")
                   for c in range(8)]
            kzt = [const.tile([96, 512], fp16, name=f"kzt{c}")
                   for c in range(8)]
            vaugT = const.tile([128, VS * NJB], fp16)
            ones16 = const.tile([1, DH], f32)
            zsb = const.tile([96, 512], fp16)
            zerob = const.tile([128, 1], f32)
            scratch1 = const.tile([128, 1], f32)
            # single PSUM bank: qkv projection staging, then the PV
            # accumulator rows 0-16 + recip-broadcast rows 64-79.
            acc_full = accp.tile([128, 512], f32)

            # --- startup: memsets, DMAs, exp-table warm ---
            nc.vector.memset(zsb[:], 0.0)
            nc.gpsimd.memset(zerob[:], 0.0)
            nc.gpsimd.memset(ones16[:], 1.0)
            # Warm the exp table set while DMAs run.
            nc.scalar.activation(scratch1[:], zerob[:], EXP, bias=zerob[:])

            # Per-block layout: col 0 = ones (denominator -> acc row 0,
            # partition-0-aligned for the DVE reciprocal), cols 1-31
            # zero, cols 32-47 = v^T (32B-aligned xbar-transpose dsts).
            va3 = vaugT[:].rearrange("p (c s) -> p c s", s=VS)
            nc.vector.memset(va3[:, :, 0:1], 1.0)
            nc.vector.memset(va3[:, :, 1:2 * DH], 0.0)

            nc.sync.dma_start(x_s[:, 0:1024], x_d.ap()[:, 0:1024])
            nc.sync.dma_start(x_s[:, 1024:2048], x_d.ap()[:, 1024:2048])
            # Zero pads for chunk 0 gate the first qk matmul: issue them
            # on sync right behind the first x chunks.
            nc.sync.dma_start(kzt[0][DH:96, :], zsb[0:80, :])
            nc.sync.dma_start(qzt[0][3 * DH:96, :], zsb[0:48, :])
            for q4 in range(2, 4):
                q4s = slice(q4 * 1024, (q4 + 1) * 1024)
                nc.sync.dma_start(x_s[:, q4s], x_d.ap()[:, q4s])
            nc.gpsimd.dma_start(w_s[:], w_d.ap())
            nc.gpsimd.dma_start(biasf[:], b_d.ap())
            nc.gpsimd.dma_start(bv_s[:], bv_d.ap())
            # Remaining zero pads ride the scalar queue, which is idle
            # until the exp stream starts.
            for c in range(1, 8):
                nc.scalar.dma_start(kzt[c][DH:96, :], zsb[0:80, :])
            for c in range(1, 8):
                nc.scalar.dma_start(qzt[c][3 * DH:96, :], zsb[0:48, :])

            # --- qkv projection: one K=128 M=48 matmul per 512-col
            # chunk, one DVE add folding all three biases, one SBUF->SBUF
            # DMA peeling k'' into the zero-padded stationary tile, one
            # xbar-transpose DMA building this chunk's v^T blocks.
            # Chunks 0-1 stage in acc_full[0:48] (freed early for the PV
            # accumulator); chunks 2-7 serialize through acc_full[64:112].
            def emit_proj(c):
                cs = slice(c * 512, (c + 1) * 512)
                if c < 2:
                    ps = acc_full[0:48, :]
                    tp = None
                else:
                    ps = acc_full[64:112, :]
                    tp = (0, 64)
                nc.tensor.matmul(ps, lhsT=w_s[:, 0:48], rhs=x_s[:, cs],
                                 start=True, stop=True, tile_position=tp)
                nc.vector.tensor_tensor(qzt[c][0:3 * DH, :], ps[:],
                                        biasf[:, cs], ADD)
                nc.gpsimd.dma_start(kzt[c][0:DH, :], qzt[c][DH:2 * DH, :])
                # [16, 512] v -> [128 (j%128), 4 (j//128), 16 (d)].
                nc.sync.dma_start_transpose(va3[:, 4 * c:4 * c + 4,
                                                2 * DH:3 * DH],
                                            qzt[c][2 * DH:3 * DH, :])

            def make_pv(pt, jbs, acc, start, stop):
                def emit():
                    for t, jb in enumerate(jbs):
                        nc.tensor.matmul(
                            acc,
                            lhsT=vaugT[:, VS * jb:VS * (jb + 1)],
                            rhs=pt[:, 512 * t:512 * (t + 1)],
                            start=(start and t == 0),
                            stop=(stop and t == len(jbs) - 1),
                            skip_group_check=True)
                return emit

            def make_tail_a(acc):
                o17 = op.tile([3 * DH, ECOLS], f32, tag="o17")
                r = op.tile([1, ECOLS], f32, tag="r")

                def emit():
                    nc.vector.tensor_copy(o17[:], acc)
                    nc.vector.reciprocal_approx_fast(r[:], o17[0:1, :])
                return emit, o17, r

            def make_tail_b(o17, r, es):
                def emit():
                    # broadcast r across 16 partitions via a ones-matmul
                    # into spare partitions of the accumulator bank.
                    bc = acc_full[64:64 + DH, :]
                    nc.tensor.matmul(bc, lhsT=ones16[:], rhs=r[:],
                                     start=True, stop=True,
                                     tile_position=(0, 64),
                                     skip_group_check=True)
                    ost = op.tile([DH, ECOLS], f32, tag="ost")
                    nc.vector.tensor_tensor(ost[:], o17[2 * DH:3 * DH, :], bc, MULT)
                    nc.vector.tensor_scalar_add(ost[:], ost[:], bv_s[:])
                    nc.sync.dma_start(o_d.ap()[:, es], ost[:])
                return emit

            from collections import deque
            pend = deque()
            pending_a = None
            pending_b = None
            acc48 = acc_full[0:3 * DH, :]

            state = {"pending_a": None, "pending_b": None}

            def emit_group(e, gi, jbs, lag):
                njb = len(jbs)
                fw = 512 * njb
                if njb == 4:
                    st = stAp.tile([128, 2048], f32, tag="A")
                else:
                    st = stBp.tile([128, 1536], f32, tag="B")
                for t, jb in enumerate(jbs):
                    kc = kzt[jb // 4][:, (jb % 4) * JW:(jb % 4 + 1) * JW]
                    nc.tensor.matmul(st[:, 512 * t:512 * (t + 1)],
                                     lhsT=kc, rhs=qzt[e][:],
                                     start=True, stop=True)
                pt = ptp.tile([128, fw], bf16, tag=("ptA" if njb == 4
                                                    else "ptB"))
                nc.scalar.activation(pt[:], st[:], EXP, bias=zerob[:])
                while len(pend) >= lag:
                    pend.popleft()()
                # The prior eighth's accumulator must be copied out
                # (tail_a) before this eighth's start=True PV clears it.
                # With lag==1 that clear is emitted at gi==1, so fire
                # tail_a at gi==0; with lag==3 it is emitted at gi==3 and
                # gi==2 keeps the copy off the PE's heels.
                tail_a_gi = 2 if lag >= 3 else 0
                if state["pending_a"] is not None and gi == tail_a_gi:
                    state["pending_a"]()
                    state["pending_a"] = None
                if state["pending_b"] is not None and gi == 6:
                    state["pending_b"]()
                    state["pending_b"] = None
                pend.append(make_pv(pt, jbs, acc48,
                                    start=(gi == 0),
                                    stop=(gi == len(GROUPS) - 1)))

            # Eighth 0 interleaves the projection chunks with the qk
            # groups so the PE FIFO never sits behind the whole
            # projection chain.  Group gi needs k'' chunks <= gi+1, so
            # keeping the emission two chunks ahead suffices.
            emit_proj(0)
            emit_proj(1)
            emit_proj(2)
            for gi, jbs in enumerate(GROUPS):
                if gi < 5:
                    emit_proj(gi + 3)
                emit_group(0, gi, jbs, LAG)
            emit_a, o17, r = make_tail_a(acc48)
            state["pending_a"] = emit_a
            state["pending_b"] = make_tail_b(o17, r, slice(0, ECOLS))

            for e in range(1, NE):
                es = slice(e * ECOLS, (e + 1) * ECOLS)
                lag = LAG if e < NE - 1 else 1
                for gi, jbs in enumerate(GROUPS):
                    emit_group(e, gi, jbs, lag)
                while pend and e == NE - 1:
                    pend.popleft()()
                emit_a, o17, r = make_tail_a(acc48)
                if state["pending_a"] is not None:
                    state["pending_a"]()
                state["pending_a"] = emit_a
                if state["pending_b"] is not None:
                    state["pending_b"]()
                state["pending_b"] = make_tail_b(o17, r, es)
            while pend:
                pend.popleft()()
            state["pending_a"]()
            state["pending_b"]()

    nc.compile()
    return nc


def _get_program():
    global _compiled
    if _compiled is None:
        _compiled = _build_program()
    return _compiled


def _prepare_core_inputs(x, w_qkv, b_qkv, emb_d, emb_h, emb_w):
    x2 = np.ascontiguousarray(
        np.asarray(x, np.float32).reshape(C, N)).astype(np.float16)
    w_qkv = np.asarray(w_qkv, np.float32)
    b_qkv = np.asarray(b_qkv, np.float32)
    scale = DH ** -0.5
    emb = (np.asarray(emb_d, np.float32)
           + np.asarray(emb_h, np.float32)
           + np.asarray(emb_w, np.float32)).reshape(DH, N)
    in_maps = []
    for h in range(NHEADS):
        qc = slice(h * DH, (h + 1) * DH)
        kc = slice(DV + h * DH, DV + (h + 1) * DH)
        vc = slice(2 * DV + h * DH, 2 * DV + (h + 1) * DH)
        w = np.empty((C, 48), np.float32)
        w[:, 0:16] = w_qkv[:, qc] * scale
        w[:, 16:32] = w_qkv[:, kc]
        w[:, 32:48] = w_qkv[:, vc]
        w = w.astype(np.float16)
        bias = np.zeros((48, N), np.float32)
        bias[0:16, :] = (b_qkv[qc] * scale)[:, None]
        bias[16:32, :] = b_qkv[kc][:, None] + emb
        bv = np.ascontiguousarray(b_qkv[vc][:, None])
        in_maps.append({"x": x2, "w": w, "bias": bias, "bv": bv})
    return in_maps


def kernel(x, w_qkv, b_qkv, emb_d, emb_h, emb_w):
    from concourse.bass_utils import run_bass_kernel_spmd

    nc = _get_program()
    in_maps = _prepare_core_inputs(x, w_qkv, b_qkv, emb_d, emb_h, emb_w)
    res = run_bass_kernel_spmd(nc, in_maps, list(range(NHEADS)))
    out = np.empty((DV, N), np.float32)
    for h in range(NHEADS):
        out[h * DH:(h + 1) * DH, :] = res.results[h]["out"]
    return out.reshape(1, DV, 16, 16, 16)
